# revision 1
# baseline (speedup 1.0000x reference)
"""Multi-head self-attention (B=2, S=2048, D=1024, H=16, causal) on 8 TRN2 NeuronCores.

Sharding: data parallel over batch (2) x tensor parallel over heads (4 groups of 4).
Core c handles batch c//4, heads 4*(c%4) .. 4*(c%4)+4.
Each core computes a partial output [2048, 1024] (its heads' contribution through
the output projection); the host sums the 4 partials per batch.

Numerics: Q/K path (projections + scores) uses bf16 hi/lo split arithmetic
(3-term products accumulated in fp32 PSUM) for ~fp32-accurate scores; softmax in
fp32 on the scalar engine (exp with fused row-sum); V / attn@V / output
projection in bf16 with fp32 accumulation.
"""
import sys
for _p in ("/opt/trn_rl_repo", "/root/.axon_site/_ro/trn_rl_repo"):
    if _p not in sys.path:
        sys.path.append(_p)

import math
from contextlib import ExitStack

import numpy as np
import ml_dtypes

import concourse.bass as bass
import concourse.bacc as bacc
import concourse.tile as tile
import concourse.mybir as mybir
from concourse.bass_utils import run_bass_kernel_spmd

BF16 = mybir.dt.bfloat16
F32 = mybir.dt.float32
SEQ = 2048
DM = 1024
DL = 256          # local head dims (4 heads x 64)
DH = 64
MC = 8            # 128-row chunks of the model dim
NQT = SEQ // 128  # 16 q tiles
NEG = -3.0e30

_CACHE = {}


def build_nc(s_bufs=5, ptp_bufs=2, av_bufs=1, work_bufs=3, small_bufs=8, nqt=NQT, pair_max=0, cw_attn=512, PTG=4, outp_bufs=2):
    nc = bacc.Bacc("TRN2", debug=False, num_devices=8)

    xh_d = nc.dram_tensor("xh", [MC, 128, SEQ], BF16, kind="ExternalInput")
    xl_d = nc.dram_tensor("xl", [MC, 128, SEQ], BF16, kind="ExternalInput")
    wqh_d = nc.dram_tensor("wqh", [MC, 128, DL], BF16, kind="ExternalInput")
    wql_d = nc.dram_tensor("wql", [MC, 128, DL], BF16, kind="ExternalInput")
    wkh_d = nc.dram_tensor("wkh", [MC, 128, DL], BF16, kind="ExternalInput")
    wkl_d = nc.dram_tensor("wkl", [MC, 128, DL], BF16, kind="ExternalInput")
    wvh_d = nc.dram_tensor("wvh", [MC, 128, DL], BF16, kind="ExternalInput")
    poT_d = nc.dram_tensor("poT", [2, 128, DM], BF16, kind="ExternalInput")
    mask_d = nc.dram_tensor("mask", [128, 128], BF16, kind="ExternalInput")
    ident_d = nc.dram_tensor("ident", [128, 128], BF16, kind="ExternalInput")
    out_d = nc.dram_tensor("out_part", [SEQ, DM], BF16, kind="ExternalOutput")

    with tile.TileContext(nc) as tc, ExitStack() as ctx:
        cst = ctx.enter_context(tc.tile_pool(name="cst", bufs=1))
        work = ctx.enter_context(tc.tile_pool(name="work", bufs=work_bufs))
        outp = ctx.enter_context(tc.tile_pool(name="outp", bufs=outp_bufs))
        small = ctx.enter_context(tc.tile_pool(name="small", bufs=small_bufs))
        sp = ctx.enter_context(tc.tile_pool(name="sp", bufs=s_bufs, space="PSUM"))
        ptp = ctx.enter_context(tc.tile_pool(name="ptp", bufs=ptp_bufs, space="PSUM"))
        avp = ctx.enter_context(tc.tile_pool(name="avp", bufs=av_bufs, space="PSUM"))

        # ---- persistent SBUF loads ----
        xh_sb = cst.tile([128, MC, SEQ], BF16, tag="xh")
        xl_sb = cst.tile([128, MC, SEQ], BF16, tag="xl")
        for m in range(MC):
            nc.sync.dma_start(out=xh_sb[:, m, :], in_=xh_d[m])
            nc.sync.dma_start(out=xl_sb[:, m, :], in_=xl_d[m])
        wsb = {}
        for nm_, d_ in (("wqh", wqh_d), ("wql", wql_d), ("wkh", wkh_d),
                        ("wkl", wkl_d), ("wvh", wvh_d)):
            t = cst.tile([128, MC, DL], BF16, tag=nm_)
            for m in range(MC):
                nc.sync.dma_start(out=t[:, m, :], in_=d_[m])
            wsb[nm_] = t
        poT_sb = cst.tile([128, 2, DM], BF16, tag="poT")
        for m in range(2):
            nc.sync.dma_start(out=poT_sb[:, m, :], in_=poT_d[m])
        mask_sb = cst.tile([128, 128], BF16, tag="mask")
        nc.sync.dma_start(out=mask_sb, in_=mask_d[:, :])
        ident_sb = cst.tile([128, 128], BF16, tag="ident")
        nc.sync.dma_start(out=ident_sb, in_=ident_d[:, :])

        # ---- projections ----
        # QT/KT: [128 part (2 d-chunks of 64+64 = heads), dc, SEQ] hi/lo bf16
        qth = cst.tile([128, 2, SEQ], BF16, tag="qth")
        qtl = cst.tile([128, 2, SEQ], BF16, tag="qtl")
        kth = cst.tile([128, 2, SEQ], BF16, tag="kth")
        ktl = cst.tile([128, 2, SEQ], BF16, tag="ktl")
        def emit_proj_chunk(qc):
            for wh_t, wl_t, oh, ol in ((wsb["wqh"], wsb["wql"], qth, qtl),
                                       (wsb["wkh"], wsb["wkl"], kth, ktl)):
                for dc in range(2):
                    ps = sp.tile([128, cw_attn], F32, tag="s", name="ps")
                    n = 0
                    for m in range(MC):
                        for lt, rt in ((wh_t, xh_sb), (wh_t, xl_sb), (wl_t, xh_sb)):
                            nc.tensor.matmul(
                                ps[:, :512],
                                lt[:, m, 128 * dc:128 * (dc + 1)],
                                rt[:, m, 512 * qc:512 * (qc + 1)],
                                start=(n == 0), stop=(n == 3 * MC - 1))
                            n += 1
                    nc.scalar.copy(out=oh[:, dc, 512 * qc:512 * (qc + 1)], in_=ps[:, :512])
                    nc.vector.tensor_sub(ol[:, dc, 512 * qc:512 * (qc + 1)],
                                         ps[:, :512], oh[:, dc, 512 * qc:512 * (qc + 1)])

        def emit_v(st):
            ps = sp.tile([128, cw_attn], F32, tag="s", name="ps")
            for m in range(MC):
                nc.tensor.matmul(ps[:, :DL], xh_sb[:, m, 128 * st:128 * (st + 1)],
                                 wsb["wvh"][:, m, :], start=(m == 0), stop=(m == MC - 1))
            nc.scalar.copy(out=v_sb[:, st, :], in_=ps[:, :DL])

        v_sb = cst.tile([128, NQT, DL], BF16, tag="v")

        # ---- attention: global-max softmax; paired S (2 heads row-packed) for
        # qi<12 in [128,1024] pair tiles, unpaired 1024-wide chunks for qi>=12 ----
        for qi in range(nqt):
            if qi % 4 == 0:
                emit_proj_chunk(qi // 4)
            emit_v(qi)
            nkt = qi + 1          # causal k tiles
            kend = nkt * 128
            attn_cat = work.tile([128, DL], BF16, tag="acat")
            for hp in range(2):
                p_sbs = [work.tile([128, SEQ], BF16, tag="p0", name="p_sb"),
                         work.tile([128, SEQ], BF16, tag="p1", name="p_sb")]
                invs = [None, None]
                paired = nkt <= pair_max
                if paired:
                    ncw = (kend + 511) // 512
                    s_tiles = [None] * ncw
                    nms = [[], []]
                    for ck in range(ncw):
                        cw = min(512, kend - 512 * ck)
                        has_mask = (ck == ncw - 1)
                        sps = sp.tile([128, 1024], F32, tag="s", name="s_ps")
                        s_tiles[ck] = sps
                        for term, (lsrc, rsrc) in enumerate(((qth, kth), (qth, ktl), (qtl, kth))):
                            for e in range(2):
                                nc.tensor.matmul(
                                    sps[:, 512 * e:512 * e + cw],
                                    lsrc[64 * e:64 * (e + 1), hp, 128 * qi:128 * (qi + 1)],
                                    rsrc[64 * e:64 * (e + 1), hp, 512 * ck:512 * ck + cw],
                                    start=(term == 0), stop=(term == 2 and not has_mask))
                        if has_mask:
                            off = (qi % 4) * 128
                            for e in range(2):
                                nc.tensor.matmul(sps[:, 512 * e + off:512 * e + off + 128],
                                                 ident_sb[:, :], mask_sb[:, :],
                                                 start=False, stop=True)
                        for e in range(2):
                            nmc = small.tile([128, 1], F32, tag="nmc", name="nmc")
                            nc.vector.tensor_reduce(out=nmc, in_=sps[:, 512 * e:512 * e + cw],
                                                    axis=mybir.AxisListType.X,
                                                    op=mybir.AluOpType.max, negate=True)
                            nms[e].append(nmc)
                    for e in range(2):
                        nm = nms[e][0]
                        for ck in range(1, ncw):
                            nmg = small.tile([128, 1], F32, tag="nmg", name="nmg")
                            nc.vector.tensor_tensor(out=nmg, in0=nm, in1=nms[e][ck],
                                                    op=mybir.AluOpType.min)
                            nm = nmg
                        total = None
                        for ck in range(ncw):
                            cw = min(512, kend - 512 * ck)
                            acc = small.tile([128, 1], F32, tag="acc", name="acc")
                            nc.scalar.activation(out=p_sbs[e][:, 512 * ck:512 * ck + cw],
                                                 in_=s_tiles[ck][:, 512 * e:512 * e + cw],
                                                 func=mybir.ActivationFunctionType.Exp,
                                                 bias=nm, scale=1.0, accum_out=acc)
                            if total is None:
                                total = acc
                            else:
                                nc.vector.tensor_add(total, total, acc)
                        inv = small.tile([128, 1], F32, tag="inv", name="inv")
                        nc.vector.reciprocal(out=inv, in_=total)
                        invs[e] = inv
                else:
                    for e in range(2):
                        CW = cw_attn
                        ncw = (kend + CW - 1) // CW
                        s_tiles = [None] * ncw
                        nms = []
                        for ck in range(ncw):
                            cw = min(CW, kend - CW * ck)
                            sps = sp.tile([128, CW], F32, tag="s", name="s_ps")
                            s_tiles[ck] = sps
                            for sub in range((cw + 511) // 512):
                                sw = min(512, cw - 512 * sub)
                                last_sub = (CW * ck + 512 * sub + sw == kend)
                                for term, (lsrc, rsrc) in enumerate(((qth, kth), (qth, ktl), (qtl, kth))):
                                    nc.tensor.matmul(
                                        sps[:, 512 * sub:512 * sub + sw],
                                        lsrc[64 * e:64 * (e + 1), hp, 128 * qi:128 * (qi + 1)],
                                        rsrc[64 * e:64 * (e + 1), hp,
                                             CW * ck + 512 * sub:CW * ck + 512 * sub + sw],
                                        start=(term == 0), stop=(term == 2 and not last_sub))
                                if last_sub:
                                    off = qi * 128 - CW * ck - 512 * sub
                                    nc.tensor.matmul(sps[:, 512 * sub + off:512 * sub + off + 128],
                                                     ident_sb[:, :], mask_sb[:, :],
                                                     start=False, stop=True)
                            nmc = small.tile([128, 1], F32, tag="nmc", name="nmc")
                            nc.vector.tensor_reduce(out=nmc, in_=sps[:, :cw],
                                                    axis=mybir.AxisListType.X,
                                                    op=mybir.AluOpType.max, negate=True)
                            nms.append(nmc)
                        nm = nms[0]
                        for ck in range(1, ncw):
                            nmg = small.tile([128, 1], F32, tag="nmg", name="nmg")
                            nc.vector.tensor_tensor(out=nmg, in0=nm, in1=nms[ck],
                                                    op=mybir.AluOpType.min)
                            nm = nmg
                        total = None
                        for ck in range(ncw):
                            cw = min(CW, kend - CW * ck)
                            acc = small.tile([128, 1], F32, tag="acc", name="acc")
                            nc.scalar.activation(out=p_sbs[e][:, CW * ck:CW * ck + cw],
                                                 in_=s_tiles[ck][:, :cw],
                                                 func=mybir.ActivationFunctionType.Exp,
                                                 bias=nm, scale=1.0, accum_out=acc)
                            if total is None:
                                total = acc
                            else:
                                nc.vector.tensor_add(total, total, acc)
                        inv = small.tile([128, 1], F32, tag="inv", name="inv")
                        nc.vector.reciprocal(out=inv, in_=total)
                        invs[e] = inv
                for e in range(2):
                    h_local = 2 * hp + e
                    p_sb = p_sbs[e]
                    # P^T via PE transpose, 8 k-tiles per group
                    pt_sb = work.tile([128, SEQ], BF16, tag=f"pt{e}", name="pt_sb")
                    for g in range((nkt + PTG - 1) // PTG):
                        n_in_g = min(PTG, nkt - PTG * g)
                        ptps = ptp.tile([128, 128 * PTG], BF16, tag="ptps", name="ptps")
                        for j in range(n_in_g):
                            kt = PTG * g + j
                            nc.tensor.transpose(ptps[:, 128 * j:128 * (j + 1)],
                                                p_sb[:, 128 * kt:128 * (kt + 1)],
                                                ident_sb)
                        if g % 2 == 0:
                            nc.vector.tensor_copy(out=pt_sb[:, 128 * PTG * g:128 * PTG * g + 128 * n_in_g],
                                                  in_=ptps[:, :128 * n_in_g])
                        else:
                            nc.scalar.copy(out=pt_sb[:, 128 * PTG * g:128 * PTG * g + 128 * n_in_g],
                                           in_=ptps[:, :128 * n_in_g])
                    # attn @ V with fp32 accumulation
                    if e == 0:
                        av_pair = avp.tile([128, 128], F32, tag="av", name="av_pair")
                    for kt in range(nkt):
                        nc.tensor.matmul(av_pair[:, 64 * e:64 * (e + 1)],
                                         pt_sb[:, 128 * kt:128 * (kt + 1)],
                                         v_sb[:, kt, 64 * h_local:64 * (h_local + 1)],
                                         start=(kt == 0), stop=(kt == nkt - 1))
                    nc.vector.tensor_scalar_mul(
                        attn_cat[:, 64 * h_local:64 * (h_local + 1)],
                        av_pair[:, 64 * e:64 * (e + 1)], invs[e])
            # ---- output projection for this q tile ----
            acT_ps = ptp.tile([128, 512], BF16, tag="ptps", name="acT_ps")
            nc.tensor.transpose(acT_ps[:, 0:128], attn_cat[:, 0:128], ident_sb)
            nc.tensor.transpose(acT_ps[:, 128:256], attn_cat[:, 128:256], ident_sb)
            acT = work.tile([128, 256], BF16, tag="acT")
            nc.scalar.copy(out=acT[:, :], in_=acT_ps[:, :256])
            out_sb = outp.tile([128, DM], BF16, tag="osb")
            for nc_i in range(2):
                ops = sp.tile([128, cw_attn], F32, tag="s", name="ops")
                for mlc in range(2):
                    nc.tensor.matmul(ops[:, :512], acT[:, 128 * mlc:128 * (mlc + 1)],
                                     poT_sb[:, mlc, 512 * nc_i:512 * (nc_i + 1)],
                                     start=(mlc == 0), stop=(mlc == 1))
                nc.vector.tensor_copy(out=out_sb[:, 512 * nc_i:512 * (nc_i + 1)], in_=ops[:, :512])
            nc.gpsimd.dma_start(out=out_d[128 * qi:128 * (qi + 1), :], in_=out_sb)

    nc.compile()
    return nc


def _bf16(a):
    return a.astype(ml_dtypes.bfloat16)


def _split(a):
    hi = _bf16(a)
    lo = _bf16(a - hi.astype(np.float32))
    return hi, lo


def _prep_inputs(x, p_q, p_k, p_v, p_o):
    """Build the 8 per-core input maps."""
    per_batch = []
    for b in range(2):
        xT = np.ascontiguousarray(x[b].T).astype(np.float32)  # [1024, 2048]
        xh, xl = _split(xT)
        per_batch.append((xh.reshape(MC, 128, SEQ), xl.reshape(MC, 128, SEQ)))

    mask = np.zeros((128, 128), np.float32)
    iu = np.triu_indices(128, 1)
    mask[iu] = NEG
    mask = _bf16(mask)
    ident = np.eye(128, dtype=ml_dtypes.bfloat16)

    per_group = []
    for g in range(4):
        rows = slice(DL * g, DL * (g + 1))
        wqT = np.ascontiguousarray((p_q[rows] / math.sqrt(DH)).T).astype(np.float32)
        wkT = np.ascontiguousarray(p_k[rows].T).astype(np.float32)
        wvT = np.ascontiguousarray(p_v[rows].T).astype(np.float32)
        poT = np.ascontiguousarray(p_o[:, rows].T).astype(np.float32)
        wqh, wql = _split(wqT)
        wkh, wkl = _split(wkT)
        per_group.append(dict(
            wqh=wqh.reshape(MC, 128, DL), wql=wql.reshape(MC, 128, DL),
            wkh=wkh.reshape(MC, 128, DL), wkl=wkl.reshape(MC, 128, DL),
            wvh=_bf16(wvT).reshape(MC, 128, DL),
            poT=_bf16(poT).reshape(2, 128, DM),
        ))

    in_maps = []
    for c in range(8):
        b, g = c // 4, c % 4
        m = dict(per_group[g])
        m["xh"], m["xl"] = per_batch[b]
        m["mask"] = mask
        m["ident"] = ident
        in_maps.append(m)
    return in_maps


def kernel(x, p_q, p_k, p_v, p_o):
    if "nc" not in _CACHE:
        _CACHE["nc"] = build_nc()
    nc = _CACHE["nc"]
    in_maps = _prep_inputs(np.asarray(x), np.asarray(p_q), np.asarray(p_k),
                           np.asarray(p_v), np.asarray(p_o))
    res = run_bass_kernel_spmd(nc, in_maps, core_ids=list(range(8)))
    parts = [r["out_part"].astype(np.float32) for r in res.results]
    out = np.stack([parts[0] + parts[1] + parts[2] + parts[3],
                    parts[4] + parts[5] + parts[6] + parts[7]])
    return out.astype(np.float32)

